# revision 37
# baseline (speedup 1.0000x reference)
"""Multi-head causal attention (b=4, n=2048, d_model=1024, 16 heads) on 8
Trainium2 NeuronCores.

Sharding: core c = (batch b = c//2, head-group hg = c%2); each core computes
one batch with 8 heads (tensor-parallel split of w_q/w_k/w_v by rows and w_o
by columns) and returns a partial [2048, 1024] output; host sums the two
head-group partials per batch.

Single interleaved instruction stream: after a short warmup + projection of
tile 0 (k, v, q), each attention q-tile runs its Act-paced exp/PV j-loops
with a queue of independent PE work (projection m-passes for the next tile,
deferred normalize and O-projection packets for earlier tiles) dispensed
between blocks so the tensor engine never stalls on the exp stream.

The causal mask is folded into the scores PSUM with an extra PE matmul
(negtri stationary x identity moving accumulates -2048 above the diagonal),
so exp produces exact zeros and no vector/pool mask op sits on the
exp -> PV dependency chain. Scores/exp/PV are column-trimmed below the
diagonal (z-offset). Softmax denominators come free from a ones column in
the PV matmul (row 64); normalization is a reciprocal + PE broadcast.

PSUM (8 banks): sp 2x[128,1024] (4) + ota/otb (2) + shared "acc" ring
[128,512] x2 (projection accumulators / O-proj / broadcast / warmup).
"""

import numpy as np

B = 4
N = 2048
D_MODEL = 1024
DK = 64
NT = 4          # q tiles of 512
QT = 512        # q tile size
KB = 128        # key block size
N_CORES = 8

_CACHE = {}


def _round_f32r(x: np.ndarray) -> np.ndarray:
    """fp16 conversion for device inputs (RNE)."""
    return np.ascontiguousarray(x, dtype=np.float32).astype(np.float16)


def _split_sync_waits(nc, max_waits=1):
    """walrus on this image allows only 1 sync-wait command per instruction;
    hoist excess waits onto same-engine NoOps inserted just before."""
    import concourse.mybir as mybir

    n_split = 0
    for fn in nc.m.functions:
        for blk in fn.blocks:
            insts = list(blk.instructions)
            out = []
            for inst in insts:
                si = inst.sync_info
                if si is not None and len(si.on_wait) > max_waits:
                    waits = list(si.on_wait)
                    head, rest = waits[:-max_waits], waits[-max_waits:]
                    while head:
                        chunk, head = head[:max_waits], head[max_waits:]
                        nop = mybir.InstNoOp(
                            name=f"{inst.name}-ws{n_split}-{len(out)}",
                            engine=inst.engine,
                            opcode="NoOp",
                            sync_info=mybir.SyncInfo(on_wait=chunk, on_update=[]),
                            bass_nofuse=True,
                        )
                        out.append(nop)
                    si.on_wait = rest
                    n_split += 1
                out.append(inst)
            if len(out) != len(insts):
                blk.instructions = out
    return n_split


def build_nc():
    import concourse.bass as bass
    import concourse.mybir as mybir
    import concourse.tile as tile
    from concourse.bass import ts

    F32 = mybir.dt.float32
    F32R = mybir.dt.float16  # compute/storage dtype for all matmul operands
    AF = mybir.ActivationFunctionType

    nc = bass.Bass("TRN2", target_bir_lowering=False, debug=False)

    qT_d = nc.dram_tensor("qT", [D_MODEL, N], F32R, kind="ExternalInput")
    kT_d = nc.dram_tensor("kT", [D_MODEL, N], F32R, kind="ExternalInput")
    vT_d = nc.dram_tensor("vT", [D_MODEL, N], F32R, kind="ExternalInput")
    wqT_d = nc.dram_tensor("wqT", [D_MODEL, 512], F32R, kind="ExternalInput")
    wkT_d = nc.dram_tensor("wkT", [D_MODEL, 512], F32R, kind="ExternalInput")
    wvT_d = nc.dram_tensor("wvT", [D_MODEL, 512], F32R, kind="ExternalInput")
    woT_d = nc.dram_tensor("woT", [512, D_MODEL], F32R, kind="ExternalInput")
    negtri_d = nc.dram_tensor("negtri", [128, 128], F32R, kind="ExternalInput")
    id2_d = nc.dram_tensor("id2", [128, 2, 128], F32R, kind="ExternalInput")
    onescol_d = nc.dram_tensor("onescol", [128, 8], F32R, kind="ExternalInput")
    sel_d = nc.dram_tensor("sel", [8, 4, 128], F32R, kind="ExternalInput")
    out_d = nc.dram_tensor("out", [N, D_MODEL], F32R, kind="ExternalOutput")

    with (
        tile.TileContext(nc) as tc,
        nc.allow_low_precision(reason="fp16 matmuls are intentional"),
    ):
        with (
            tc.tile_pool(name="persist", bufs=1) as persist,
            tc.tile_pool(name="pt_pool", bufs=1) as pt_pool,
            tc.tile_pool(name="outp", bufs=1) as outp,
            tc.tile_pool(name="xsp", bufs=1) as xsp,
            tc.tile_pool(name="ps", bufs=1, space="PSUM") as ps,
        ):
            qT_all = persist.tile([128, 4, N], F32R)   # [part, m-block, seq]
            kT_all = persist.tile([128, 4, N], F32R)
            v_all = persist.tile([128, 16, 8, 65], F32R)  # [k-part, sb, head, d+1]
            wq_sb = persist.tile([128, 8, 512], F32R)
            wk_sb = persist.tile([128, 8, 512], F32R)
            wv_sb = persist.tile([128, 8, 512], F32R)
            wo_sb = persist.tile([128, 4, D_MODEL], F32R)
            negtri_sb = persist.tile([128, 128], F32R)
            id2_sb = persist.tile([128, 2, 128], F32R)
            onescol_sb = persist.tile([128, 8], F32R)
            sel_sb = persist.tile([8, 4, 128], F32R)
            junk = persist.tile([128, 640], F32R)
            ot_sb = [
                persist.tile([128, 4, QT], F32R, name=f"ot_sb{t}", tag=f"ot{t}")
                for t in range(NT)
            ]
            rs_sb = [
                persist.tile([8, QT], F32R, name=f"rs_sb{t}", tag=f"rs{t}")
                for t in range(NT)
            ]
            recip_sb = [
                persist.tile([8, QT], F32R, name=f"recip{t}", tag=f"rc{t}")
                for t in range(NT)
            ]

            # weight/constant DMAs: interleaved with the tile-0 x-loads so the
            # k-unit's inputs (wk + its x8) land first and the fill isn't
            # starved behind 4MB of weights
            nc.vector.memset(junk, 0.0)
            # tile 3's normalize runs in two stages; zero its reciprocal tile
            # so the first-stage K=8 broadcast never reads uninitialized rows
            nc.vector.memset(recip_sb[3], 0.0)
            # m0 column chunks first: they gate the very first attention
            # blocks (k/q m0 passes), the remaining columns can trail
            for kc in range(8):
                nc.sync.dma_start(
                    out=wk_sb[:, kc, 0:128], in_=wkT_d[ts(kc, 128), 0:128]
                )
            for kc in range(8):
                nc.gpsimd.dma_start(
                    out=wq_sb[:, kc, 0:128], in_=wqT_d[ts(kc, 128), 0:128]
                )
            for kc in range(8):
                nc.scalar.dma_start(out=wv_sb[:, kc, :], in_=wvT_d[ts(kc, 128), :])
            nc.gpsimd.dma_start(out=negtri_sb, in_=negtri_d[:, :])
            nc.gpsimd.dma_start(out=id2_sb, in_=id2_d[:, :, :])
            nc.gpsimd.dma_start(out=onescol_sb, in_=onescol_d[:, :])
            nc.gpsimd.dma_start(out=sel_sb, in_=sel_d[:, :, :])

            # ---------------- projection unit packets ----------------
            def proj_unit(src_d, w_sb, kind, t, dma_eng=None):
                """Packets: 1 DMA packet, then per m: 4 matmul packets + copy.
                m-major over a resident [128, 8, QT] x-tile so each input
                tile is DMA'd exactly once. DMA triggers go through the given
                engine's DGE queue (sync's ~650ns/trigger rate is the fill
                bottleneck otherwise)."""
                x8 = [None]
                eng = dma_eng if dma_eng is not None else nc.gpsimd

                def dmas():
                    x8[0] = xsp.tile(
                        [128, 8, QT], F32R, name="x8", tag="x8", bufs=3
                    )
                    for kc in range(8):
                        eng.dma_start(
                            out=x8[0][:, kc, :], in_=src_d[ts(kc, 128), ts(t, QT)]
                        )

                yield dmas
                for m in range(4):
                    st = [None]

                    def mk_mm(m, kcs, first, st=st):
                        def pkt():
                            if first:
                                st[0] = ps.tile(
                                    [128, QT], F32, name="acc", tag="acc", bufs=2
                                )
                            for kc in kcs:
                                if kind == "qk":
                                    nc.tensor.matmul(
                                        st[0],
                                        w_sb[:, kc, ts(m, 128)],
                                        x8[0][:, kc, :],
                                        start=(kc == 0),
                                        stop=(kc == 7),
                                    )
                                else:
                                    nc.tensor.matmul(
                                        st[0],
                                        x8[0][:, kc, ts(m, 128)],
                                        w_sb[:, kc, :],
                                        start=(kc == 0),
                                        stop=(kc == 7),
                                    )

                        return pkt

                    yield mk_mm(m, (0, 1), True)
                    yield mk_mm(m, (2, 3), False)
                    yield mk_mm(m, (4, 5), False)
                    yield mk_mm(m, (6, 7), False)

                    def copy_out(m=m, st=st):
                        if kind == "qk":
                            dst = qT_all if src_d is qT_d else kT_all
                            nc.vector.tensor_copy(dst[:, m, ts(t, QT)], st[0])
                        else:
                            sb = t * 4 + m
                            nc.vector.tensor_copy(
                                v_all[:, sb, :, 0:64],
                                st[0][:, :].rearrange("p (h d) -> p h d", h=8),
                            )
                            nc.vector.tensor_copy(v_all[:, sb, :, 64], onescol_sb)

                    yield copy_out

            # ---------------- normalize / O-proj packets ----------------
            def norm_packets(t):
                def rcp():
                    nc.vector.reciprocal(recip_sb[t], rs_sb[t])

                yield rcp
                for g in range(4):

                    def bcmul(g=g):
                        bc = ps.tile([128, QT], F32, name="acc", tag="acc", bufs=2)
                        nc.tensor.matmul(
                            bc,
                            sel_sb[:, g, :],
                            recip_sb[t][:, :],
                            start=True, stop=True,
                        )
                        nc.vector.tensor_mul(
                            ot_sb[t][:, g, :], ot_sb[t][:, g, :], bc
                        )

                    yield bcmul

            def oproj_packets(t, use_act=False):
                for mm in range(4):
                    for n2 in range(2):

                        def po_pkt(mm=mm, n2=n2):
                            po = ps.tile(
                                [128, 512], F32, name="acc", tag="acc", bufs=2
                            )
                            for g in range(4):
                                nc.tensor.matmul(
                                    po,
                                    ot_sb[t][:, g, ts(mm, 128)],
                                    wo_sb[:, g, ts(n2, 512)],
                                    start=(g == 0),
                                    stop=(g == 3),
                                )
                            ob = outp.tile(
                                [128, 512], F32R, name="ob", tag="ob", bufs=3
                            )
                            if use_act:
                                nc.scalar.copy(ob, po)
                                # tail: split across four chunks on all three
                                # trigger-capable engines so the final
                                # transfers drain ~4x faster (~6GB/s/queue)
                                for ci, eng in enumerate(
                                    (nc.sync, nc.scalar, nc.gpsimd, nc.sync)
                                ):
                                    lo = n2 * 512 + ci * 128
                                    eng.dma_start(
                                        out=out_d[
                                            ts(4 * t + mm, 128), lo : lo + 128
                                        ],
                                        in_=ob[:, ci * 128 : ci * 128 + 128],
                                    )
                            else:
                                nc.vector.tensor_copy(ob, po)
                                nc.sync.dma_start(
                                    out=out_d[ts(4 * t + mm, 128), ts(n2, 512)],
                                    in_=ob,
                                )

                        yield po_pkt

            # ---------------- attention tile with dispenser ----------------
            def emit_scores(t, g, j, sp):
                r = j - 4 * t
                z = 128 * r if r > 0 else 0
                nc.tensor.matmul(
                    sp[:, z:QT],
                    kT_all[0:64, g, ts(j, 128)],
                    qT_all[0:64, g, t * QT + z : (t + 1) * QT],
                    start=True,
                    stop=True,
                    tile_position=(0, 0),
                )
                nc.tensor.matmul(
                    sp[:, QT + z : 2 * QT],
                    kT_all[64:128, g, ts(j, 128)],
                    qT_all[64:128, g, t * QT + z : (t + 1) * QT],
                    start=True,
                    stop=True,
                    tile_position=(64, 0),
                )
                if r >= 0:
                    # fold the causal mask into the scores: accumulate -2048
                    # above the diagonal of the 128-wide window so exp -> 0
                    sp3 = sp.rearrange("p (h q) -> p h q", h=2)
                    nc.tensor.matmul(
                        sp3[:, :, z : z + 128],
                        negtri_sb,
                        id2_sb,
                        start=False,
                        stop=True,
                        skip_group_check=True,
                    )

            def attn_tile(t, queue, late_queue=()):
                nkb = 4 * t + 4  # causal: key blocks 0 .. 4t+3
                nblocks = nkb * 4
                total = len(queue)
                dispensed = 0
                late = list(late_queue)
                bi = 0
                for g in range(4):
                    ota = ps.tile([65, QT], F32, name="ota", tag="ota", bufs=1)
                    otb = ps.tile([65, QT], F32, name="otb", tag="otb", bufs=1)
                    sp_cur = ps.tile(
                        [128, 2 * QT], F32, name="sp", tag="sp", bufs=2
                    )
                    emit_scores(t, g, 0, sp_cur)
                    for j in range(nkb):
                        r = j - 4 * t
                        z = 128 * r if r > 0 else 0
                        sp = sp_cur
                        if j + 1 < nkb:
                            sp_cur = ps.tile(
                                [128, 2 * QT], F32, name="sp", tag="sp", bufs=2
                            )
                            emit_scores(t, g, j + 1, sp_cur)
                        pt2 = pt_pool.tile(
                            [128, 2 * QT], F32R, name="pt2", tag="pt2", bufs=6
                        )
                        if z:
                            sp3 = sp.rearrange("p (h q) -> p h q", h=2)
                            pt3 = pt2.rearrange("p (h q) -> p h q", h=2)
                            nc.scalar.activation(
                                pt3[:, :, z:QT], sp3[:, :, z:QT], AF.Exp,
                                scale=0.125,
                            )
                        else:
                            nc.scalar.activation(pt2, sp, AF.Exp, scale=0.125)
                        bi += 1
                        want = total * bi // nblocks
                        while dispensed < want:
                            queue[dispensed]()
                            dispensed += 1
                        if g == 3 and late and j >= 2 and j % 3 == 2:
                            late.pop(0)()
                        nc.tensor.matmul(
                            ota[:, z:QT],
                            v_all[:, j, 2 * g, :],
                            pt2[:, z:QT],
                            start=(j == 0),
                            stop=(j == nkb - 1),
                        )
                        nc.tensor.matmul(
                            otb[:, z:QT],
                            v_all[:, j, 2 * g + 1, :],
                            pt2[:, QT + z : 2 * QT],
                            start=(j == 0),
                            stop=(j == nkb - 1),
                        )
                    # epilogue: stage O^T + rowsums to SBUF. For the very
                    # last group the rowsum copies go to the (idle) Act
                    # engine so the tail's reciprocal chain starts sooner.
                    nc.vector.tensor_copy(ot_sb[t][0:64, g, :], ota[0:64, :])
                    nc.vector.tensor_copy(ot_sb[t][64:128, g, :], otb[0:64, :])
                    tmp_rs = pt_pool.tile(
                        [1, 2, QT], F32R, name="tmp_rs", tag="tmp_rs", bufs=2
                    )
                    cp = nc.scalar.copy if (t == 3 and g == 3) else (
                        nc.vector.tensor_copy
                    )
                    cp(tmp_rs[0:1, 0, :], ota[64:65, :])
                    cp(tmp_rs[0:1, 1, :], otb[64:65, :])
                    nc.sync.dma_start(
                        out=rs_sb[t][2 * g : 2 * g + 2, :], in_=tmp_rs[0:1, :, :]
                    )
                while dispensed < total:
                    queue[dispensed]()
                    dispensed += 1
                for pkt in late:
                    pkt()

            def norm3_part1():
                # groups 0-2 of tile 3: their rowsums are final once group 2's
                # epilogue lands, so normalize them during group 3's j-loop
                def rcp():
                    nc.vector.reciprocal(recip_sb[3][0:6, :], rs_sb[3][0:6, :])

                yield rcp
                for pkt in list(norm_packets(3))[1:4]:
                    yield pkt

            def norm3_part2():
                def rcp():
                    # recomputes rows 0:6 harmlessly; DVE reciprocal cost is
                    # free-size-bound and partition bases must be aligned
                    nc.vector.reciprocal(recip_sb[3], rs_sb[3])

                yield rcp
                yield list(norm_packets(3))[4]

            # ---------------- main schedule ----------------
            # fill: issue tile-0 x-loads interleaved with the remaining
            # weight DMAs, warm up the PE, then drain tile-0's proj packets
            units0 = [
                list(proj_unit(kT_d, wk_sb, "qk", 0, dma_eng=nc.sync)),
                list(proj_unit(vT_d, wv_sb, "v", 0, dma_eng=nc.scalar)),
                list(proj_unit(qT_d, wq_sb, "qk", 0, dma_eng=nc.gpsimd)),
            ]
            units0[0][0]()  # k0 x8 DMAs
            units0[1][0]()  # v0 x8 DMAs
            units0[2][0]()  # q0 x8 DMAs
            for kc in range(8):
                nc.sync.dma_start(
                    out=wk_sb[:, kc, 128:512], in_=wkT_d[ts(kc, 128), 128:512]
                )
            for kc in range(8):
                nc.gpsimd.dma_start(
                    out=wq_sb[:, kc, 128:512], in_=wqT_d[ts(kc, 128), 128:512]
                )
            # wo is first needed by oproj(0) inside attn(2) - load it last
            for g in range(4):
                nc.gpsimd.dma_start(out=wo_sb[:, g, :], in_=woT_d[ts(g, 128), :])

            pwarm = ps.tile([128, QT], F32, name="acc", tag="acc", bufs=2)
            for _ in range(8):
                nc.tensor.matmul(
                    pwarm[:, 0:320], junk[:, 0:128], junk[:, 128:448],
                    start=True, stop=True,
                )
            # fill: only the m0 passes of k/q plus all of v, then start
            # attention on tile 0 with the remaining passes in its queue so
            # the Act engine starts ~15us earlier
            k0, v0, q0 = units0
            for pkt in k0[1:6]:
                pkt()
            for pkt in v0[1:]:
                pkt()
            for pkt in q0[1:6]:
                pkt()

            # normalize packets sit mid-queue: the 3.3us reciprocal needs
            # slack before its dependent bc matmuls enter the PE stream
            def tile_queue(t, prev_norm):
                u = [
                    list(proj_unit(kT_d, wk_sb, "qk", t, dma_eng=nc.sync)),
                    list(proj_unit(vT_d, wv_sb, "v", t, dma_eng=nc.scalar)),
                    list(proj_unit(qT_d, wq_sb, "qk", t, dma_eng=nc.gpsimd)),
                ]
                return u[0] + prev_norm + u[1] + u[2]

            attn_tile(0, k0[6:] + q0[6:] + tile_queue(1, []))
            attn_tile(1, tile_queue(2, list(norm_packets(0))))
            attn_tile(2, tile_queue(3, list(norm_packets(1))))
            attn_tile(
                3,
                list(oproj_packets(0))
                + list(norm_packets(2))
                + list(oproj_packets(1))
                + list(oproj_packets(2)),
                late_queue=list(norm3_part1()),
            )
            for pkt in norm3_part2():
                pkt()
            for pkt in oproj_packets(3, use_act=True):
                pkt()

    _split_sync_waits(nc)
    return nc


def _prep_inputs(Q, K, V, w_q, w_k, w_v, w_o):
    """Build the 8 per-core input maps (host-side shard + transpose + f32r)."""
    Q = np.asarray(Q, dtype=np.float32)
    K = np.asarray(K, dtype=np.float32)
    V = np.asarray(V, dtype=np.float32)
    w_q = np.asarray(w_q, dtype=np.float32)
    w_k = np.asarray(w_k, dtype=np.float32)
    w_v = np.asarray(w_v, dtype=np.float32)
    w_o = np.asarray(w_o, dtype=np.float32)

    # causal-mask matmul constants: negtri (stationary) accumulates -2048
    # above the diagonal of the 128-wide window via the duplicated identity
    # (moving), making exp of masked scores exactly 0
    p_idx = np.arange(128)[:, None]
    k_idx = np.arange(128)[None, :]
    negtri = np.where(k_idx > p_idx, np.float16(-2048.0), np.float16(0.0))
    id2 = np.zeros((128, 2, 128), dtype=np.float16)
    eye = np.eye(128, dtype=np.float16)
    id2[:, 0, :] = eye
    id2[:, 1, :] = eye
    onescol = np.ones((128, 8), dtype=np.float16)
    sel = np.zeros((8, 4, 128), dtype=np.float16)
    for g in range(4):
        sel[2 * g, g, 0:64] = 1.0
        sel[2 * g + 1, g, 64:128] = 1.0

    qT = [_round_f32r(Q[b].T) for b in range(B)]
    kT = [_round_f32r(K[b].T) for b in range(B)]
    vT = [_round_f32r(V[b].T) for b in range(B)]
    wqT = [_round_f32r(w_q[hg * 512 : hg * 512 + 512, :].T) for hg in range(2)]
    wkT = [_round_f32r(w_k[hg * 512 : hg * 512 + 512, :].T) for hg in range(2)]
    wvT = [_round_f32r(w_v[hg * 512 : hg * 512 + 512, :].T) for hg in range(2)]
    woT = [_round_f32r(w_o[:, hg * 512 : hg * 512 + 512].T) for hg in range(2)]

    in_maps = []
    for c in range(N_CORES):
        b, hg = c // 2, c % 2
        in_maps.append(
            {
                "qT": qT[b],
                "kT": kT[b],
                "vT": vT[b],
                "wqT": wqT[hg],
                "wkT": wkT[hg],
                "wvT": wvT[hg],
                "woT": woT[hg],
                "negtri": negtri,
                "id2": id2,
                "onescol": onescol,
                "sel": sel,
            }
        )
    return in_maps


def kernel(Q, K, V, w_q, w_k, w_v, w_o, _trace=False):
    from concourse.bass_utils import run_bass_kernel_spmd

    if "nc" not in _CACHE:
        _CACHE["nc"] = build_nc()
    nc = _CACHE["nc"]

    in_maps = _prep_inputs(Q, K, V, w_q, w_k, w_v, w_o)
    res = run_bass_kernel_spmd(
        nc, in_maps, core_ids=list(range(N_CORES)), trace=_trace
    )
    outs = [r["out"] for r in res.results]
    full = np.empty((B, N, D_MODEL), dtype=np.float32)
    for b in range(B):
        full[b] = outs[2 * b].astype(np.float32) + outs[2 * b + 1].astype(
            np.float32
        )
    if _trace:
        _CACHE["last_result"] = res
    return full
